# revision 2
# baseline (speedup 1.0000x reference)
"""Trainium2 Bass kernel for nn_MultiHeadCrossAttention (B=32, Nc=2048, H=8, topk=12).

D4 design: single-fp16-term S matmuls; ACT drains S chunks to fp16 SBUF;
DVE pair-max + 3-level tournament -> T[1024]; fp32 pack pm = q10*1024 + slot;
quarter max8 + match_replace rounds extract top-12 with indices; leaf/parity
resolved via group-redundant indirect_copy + mask-reduce; V never materialized:
gather comp columns (fp32), weight, scatter_add into M[(e,r)], finish with 8
accumulated matmuls Wv_e^T @ M_e, diagonal-extract via hrep mask.
"""

import sys
import numpy as np

for p in ("/opt/trn_rl_repo",):
    if p not in sys.path:
        sys.path.insert(0, p)

import ml_dtypes

B, CORES, BPC = 32, 8, 4
H, HD, NQ, TK, C, NC = 8, 16, 16, 12, 128, 2048
NJ = 8 * NC
MAGIC = 12582912.0          # 2^23 + 2^22
BIG = 3.0 * 2.0**32         # MAGIC * 1024: rounds fp32 to multiples of 1024
PACK_MUL = 8192.0 * 1024.0  # value quantum 1/8192 over range (-1, 1)
PACK_ADD = (8192.0 + MAGIC) * 1024.0
PACK_SUB = BIG
LQ_SCALE = 8192.0
LQ_BIAS = 8192.0 + MAGIC
NEG = -1e30

_prog_cache = {}


def _build_program():
    import concourse.bass as bass
    import concourse.mybir as mybir
    import concourse.tile as tile
    from concourse import bacc
    from concourse import library_config

    dt = mybir.dt
    Alu = mybir.AluOpType
    Act = mybir.ActivationFunctionType
    f32, f16, bf16 = dt.float32, dt.float16, dt.bfloat16
    nc = bacc.Bacc("TRN2", target_bir_lowering=False)

    c16h_d = nc.dram_tensor("c16h", [BPC, C, NC], f16, kind="ExternalInput")
    c32_d = nc.dram_tensor("c32", [BPC, C, NC], f32, kind="ExternalInput")
    xT_d = nc.dram_tensor("xT", [C, BPC], f32, kind="ExternalInput")
    wq_d = nc.dram_tensor("wq", [C, 2048], f32, kind="ExternalInput")
    wkT_d = nc.dram_tensor("wkT", [C, 8 * C], f16, kind="ExternalInput")
    wv_d = nc.dram_tensor("wv", [C, 8 * C], bf16, kind="ExternalInput")
    wjwp_d = nc.dram_tensor("wjwp", [C, NQ * C], f32, kind="ExternalInput")
    wp_d = nc.dram_tensor("wp", [C, C], f32, kind="ExternalInput")
    bp4_d = nc.dram_tensor("bp4", [BPC, C], f32, kind="ExternalInput")
    hrep_d = nc.dram_tensor("hrep", [C, C], f16, kind="ExternalInput")
    ident_d = nc.dram_tensor("ident", [C, C], f32, kind="ExternalInput")
    srow_d = nc.dram_tensor("srow", [C, 1024], f32, kind="ExternalInput")
    krow_d = nc.dram_tensor("krow", [C, 96], f32, kind="ExternalInput")
    kval_d = nc.dram_tensor("kval", [C, 96], f32, kind="ExternalInput")
    dsel_d = nc.dram_tensor("dsel", [C, 1536], f16, kind="ExternalInput")
    rcon_d = nc.dram_tensor("rcon", [C, 1], f32, kind="ExternalInput")
    scr_d = [
        nc.dram_tensor(f"scr{b}", [36, C], f16, kind="Internal")
        for b in range(BPC)
    ]
    out_d = nc.dram_tensor("out", [BPC, C], f32, kind="ExternalOutput")

    with tile.TileContext(nc) as tc:
        nc.gpsimd.load_library(library_config.ap_gather)
        with (
            tc.tile_pool(name="weights", bufs=1) as wpool,
            tc.tile_pool(name="inb", bufs=2) as inpool,       # c16h, c32
            tc.tile_pool(name="sbig", bufs=2) as sbig,        # SbEven, R
            tc.tile_pool(name="stage", bufs=3) as stpool,     # odd chunks
            tc.tile_pool(name="mid", bufs=1) as midpool,      # L1, L2, pm, G, M...
            tc.tile_pool(name="small", bufs=2) as smpool,     # winner stage tiles
            tc.tile_pool(name="ps_s", bufs=2, space="PSUM") as ps_s,
            tc.tile_pool(name="ps_a", bufs=1, space="PSUM") as ps_a,
            tc.tile_pool(name="ps_m", bufs=1, space="PSUM") as ps_m,
            tc.tile_pool(name="ps_pv", bufs=1, space="PSUM") as ps_pv,
        ):
            # ---- resident weights/constants ----
            wq_s = wpool.tile([C, 2048], f32)
            nc.sync.dma_start(wq_s[:], wq_d[:])
            wkT_s = wpool.tile([C, 8 * C], f16)
            nc.sync.dma_start(wkT_s[:], wkT_d[:])
            wv_s = wpool.tile([C, 8 * C], bf16)
            nc.sync.dma_start(wv_s[:], wv_d[:])
            wjwp_s = wpool.tile([C, NQ * C], f32)
            nc.sync.dma_start(wjwp_s[:], wjwp_d[:])
            wp_s = wpool.tile([C, C], f32)
            nc.sync.dma_start(wp_s[:], wp_d[:])
            bp4_s = wpool.tile([BPC, C], f32)
            nc.sync.dma_start(bp4_s[:], bp4_d[:])
            hrep_s = wpool.tile([C, C], f16)
            nc.sync.dma_start(hrep_s[:], hrep_d[:])
            ident_s = wpool.tile([C, C], f32)
            nc.sync.dma_start(ident_s[:], ident_d[:])
            srow_s = wpool.tile([C, 1024], f32)
            nc.sync.dma_start(srow_s[:], srow_d[:])
            krow_s = wpool.tile([C, 96], f32)
            nc.sync.dma_start(krow_s[:], krow_d[:])
            kval_s = wpool.tile([C, 96], f32)
            nc.sync.dma_start(kval_s[:], kval_d[:])
            dsel_s = wpool.tile([C, 1536], f16)
            nc.sync.dma_start(dsel_s[:], dsel_d[:])
            rcon_s = wpool.tile([C, 1], f32)
            nc.sync.dma_start(rcon_s[:], rcon_d[:])
            xT_s = wpool.tile([C, BPC], f32)
            nc.sync.dma_start(xT_s[:], xT_d[:])

            gwe_s = wpool.tile([C, 3072], bf16)  # padded Gw, odd slots stay 0
            nc.vector.memset(gwe_s[:], 0.0)
            bm1_s = wpool.tile([C, 1], f32)      # exp bias constant
            nc.vector.memset(bm1_s[:], -1.0)

            # ---- Q projection for all batches: qt [(h,hd), (q,b)] ----
            qt_ps = ps_m.tile([C, 512], f32, tag="misc")
            for qi in range(NQ):
                nc.tensor.matmul(
                    qt_ps[:, qi * BPC:(qi + 1) * BPC],
                    wq_s[:, qi * C:(qi + 1) * C],
                    xT_s[:],
                )
            qt_s = wpool.tile([C, NQ * BPC], f32)
            nc.scalar.copy(qt_s[:], qt_ps[:, : NQ * BPC])

            pvt4_s = wpool.tile([C, NQ * BPC], f32)

            for b in range(BPC):
                c16 = inpool.tile([C, NC], f16, tag="c16")
                nc.sync.dma_start(c16[:], c16h_d[b])
                c32 = inpool.tile([C, NC], f32, tag="c32")
                nc.sync.dma_start(c32[:], c32_d[b])

                # ---- qbd (block diag, 0.25 scale), fp16 ----
                qfull_s = smpool.tile([C, C], f32, tag="qfull")
                qsl = (
                    qt_s[:, b::BPC]
                    .rearrange("p (o q) -> p o q", o=1)
                    .to_broadcast([C, H, NQ])
                )
                nc.vector.tensor_scalar(
                    qfull_s[:].rearrange("p (o q) -> p o q", o=H),
                    qsl, 0.25, None, Alu.mult,
                )
                qbd_s = smpool.tile([C, C], f16, tag="qbd")
                nc.vector.tensor_mul(qbd_s[:], qfull_s[:], hrep_s[:])

                # ---- A_e [c, row] fp16 ----
                a16 = smpool.tile([C, 8 * C], f16, tag="a16")
                for half in range(2):
                    a_ps = ps_a.tile([C, 512], f32, tag="a")
                    for i in range(4):
                        e = half * 4 + i
                        nc.tensor.matmul(
                            a_ps[:, i * C:(i + 1) * C],
                            wkT_s[:, e * C:(e + 1) * C],
                            qbd_s[:],
                        )
                    nc.scalar.copy(a16[:, half * 512:(half + 1) * 512], a_ps[:])

                # ---- S chunks -> ACT fp16 copies; L0 pair-max -> R ----
                sbe = sbig.tile([C, 8192], f16, tag="sbe")   # even chunks
                r_s = sbig.tile([C, 8192], f16, tag="R")
                for k in range(8):   # chunk pair k: chunks 2k (even), 2k+1 (odd)
                    odd = stpool.tile([C, 1024], f16, tag="odd")
                    for ch in (2 * k, 2 * k + 1):
                        e, half = ch // 2, ch % 2
                        s_ps = ps_s.tile([C, 1024], f32, tag="s")
                        for n2 in range(2):
                            col = half * 1024 + n2 * 512
                            nc.tensor.matmul(
                                s_ps[:, n2 * 512:(n2 + 1) * 512],
                                a16[:, e * C:(e + 1) * C],
                                c16[:, col:col + 512],
                            )
                        dst = sbe[:, k * 1024:(k + 1) * 1024] if ch % 2 == 0 else odd[:]
                        nc.scalar.copy(dst, s_ps[:])
                    nc.vector.tensor_max(
                        r_s[:, k * 1024:(k + 1) * 1024],
                        sbe[:, k * 1024:(k + 1) * 1024],
                        odd[:],
                    )

                # ---- tournament: R [p,8,1024] -> T [p,1024] ----
                l1_s = midpool.tile([C, 4096], f16, tag="L1")
                rv = r_s[:].rearrange("p (k f) -> p k f", k=8)
                nc.vector.tensor_max(
                    l1_s[:].rearrange("p (k f) -> p k f", k=4),
                    rv[:, 0:4, :], rv[:, 4:8, :],
                )
                l2_s = midpool.tile([C, 2048], f16, tag="L2")
                l1v = l1_s[:].rearrange("p (k f) -> p k f", k=4)
                nc.vector.tensor_max(
                    l2_s[:].rearrange("p (k f) -> p k f", k=2),
                    l1v[:, 0:2, :], l1v[:, 2:4, :],
                )
                t_s = midpool.tile([C, 1024], f16, tag="T")
                nc.vector.tensor_max(t_s[:], l2_s[:, 0:1024], l2_s[:, 1024:2048])

                # ---- pack pm = q10*1024 + s (exact fp32 ints) ----
                t1_s = midpool.tile([C, 1024], f32, tag="t1")
                nc.vector.tensor_scalar(
                    t1_s[:], t_s[:], PACK_MUL, PACK_ADD, Alu.mult, Alu.add
                )
                pm_s = midpool.tile([C, 1024], f32, tag="pm")
                nc.vector.scalar_tensor_tensor(
                    pm_s[:], t1_s[:], PACK_SUB, srow_s[:], Alu.subtract, Alu.add
                )

                # ---- quarter extract -> 32 cands -> top8 + next4 ----
                cand_s = smpool.tile([C, 32], f32, tag="cand")
                for qd in range(4):
                    nc.vector.max(
                        cand_s[:, qd * 8:(qd + 1) * 8],
                        pm_s[:, qd * 256:(qd + 1) * 256],
                    )
                t8a = smpool.tile([C, 8], f32, tag="t8a")
                nc.vector.max(t8a[:], cand_s[:])
                c2_s = smpool.tile([C, 32], f32, tag="c2")
                nc.vector.match_replace(c2_s[:], t8a[:], cand_s[:], NEG)
                t8b = smpool.tile([C, 8], f32, tag="t8b")
                nc.vector.max(t8b[:], c2_s[:])
                pw_s = smpool.tile([C, 12], f32, tag="pw")
                nc.vector.tensor_copy(pw_s[:, 0:8], t8a[:])
                nc.vector.tensor_copy(pw_s[:, 8:12], t8b[:, 0:4])

                # ---- decode: r1 = round1024(pm); s = pm - r1 (mod fix); qv ----
                r1_s = smpool.tile([C, 12], f32, tag="r1")
                nc.vector.tensor_scalar(
                    r1_s[:], pw_s[:], BIG, BIG, Alu.add, Alu.subtract
                )
                sp_s = smpool.tile([C, 12], f32, tag="sp")
                nc.vector.tensor_sub(sp_s[:], pw_s[:], r1_s[:])
                neg_s = smpool.tile([C, 12], f32, tag="neg")
                nc.vector.tensor_scalar(neg_s[:], sp_s[:], 0.0, None, Alu.is_lt)
                s_sl = smpool.tile([C, 12], f32, tag="s")
                nc.vector.scalar_tensor_tensor(
                    s_sl[:], neg_s[:], 1024.0, sp_s[:], Alu.mult, Alu.add
                )
                qv_s = smpool.tile([C, 12], f32, tag="qv")
                nc.vector.scalar_tensor_tensor(
                    qv_s[:], r1_s[:], 1.0 / 1024.0, neg_s[:], Alu.mult, Alu.subtract
                )
                g0_s = smpool.tile([C, 12], f32, tag="g0")
                nc.vector.tensor_scalar(g0_s[:], qv_s[:], MAGIC, None, Alu.add)

                # ---- softmax weights from quantized values ----
                expv_s = smpool.tile([C, 12], f32, tag="expv")
                nc.scalar.activation(
                    expv_s[:], qv_s[:], Act.Exp, bias=bm1_s[:], scale=1.0 / 8192.0
                )
                den_s = smpool.tile([C, 1], f32, tag="den")
                nc.vector.tensor_reduce(
                    den_s[:], expv_s[:], mybir.AxisListType.X, Alu.add
                )
                rden_s = smpool.tile([C, 1], f32, tag="rden")
                nc.vector.reciprocal(rden_s[:], den_s[:])

                # ---- leaf resolve: gather 8 leaves/winner (group redundant) ----
                i1_s = smpool.tile([C, 96], f32, tag="i1")
                nc.vector.tensor_add(
                    i1_s[:].rearrange("p (w k) -> p w k", w=12),
                    krow_s[:].rearrange("p (w k) -> p w k", w=12),
                    s_sl[:].rearrange("p (w o) -> p w o", o=1).to_broadcast([C, 12, 8]),
                )
                i1u_s = smpool.tile([C, 96], dt.uint16, tag="i1u")
                nc.vector.tensor_copy(i1u_s[:], i1_s[:])
                g1_s = midpool.tile([C, 1536], f16, tag="g1")
                nc.gpsimd.indirect_copy(
                    g1_s[:, 0:768], r_s[:], i1u_s[:, 0:48], True
                )
                nc.gpsimd.indirect_copy(
                    g1_s[:, 768:1536], r_s[:], i1u_s[:, 48:96], True
                )
                g1m_s = midpool.tile([C, 1536], f16, tag="g1m")
                nc.vector.tensor_mul(g1m_s[:], g1_s[:], dsel_s[:])
                diag_s = smpool.tile([C, 96], f32, tag="diag")
                nc.vector.tensor_reduce(
                    diag_s[:],
                    g1m_s[:].rearrange("p (t b) -> p t b", t=96),
                    mybir.AxisListType.X, Alu.add,
                )
                lq_s = smpool.tile([C, 96], f32, tag="lq")
                nc.vector.tensor_scalar(
                    lq_s[:], diag_s[:], LQ_SCALE, LQ_BIAS, Alu.mult, Alu.add
                )
                eq_s = smpool.tile([C, 96], f32, tag="eq")
                nc.vector.tensor_tensor(
                    eq_s[:].rearrange("p (w k) -> p w k", w=12),
                    lq_s[:].rearrange("p (w k) -> p w k", w=12),
                    g0_s[:].rearrange("p (w o) -> p w o", o=1).to_broadcast([C, 12, 8]),
                    Alu.is_equal,
                )
                kk_s = smpool.tile([C, 96], f32, tag="kk")
                nc.vector.tensor_mul(kk_s[:], eq_s[:], kval_s[:])
                kp1_s = smpool.tile([C, 12], f32, tag="kp1")
                nc.vector.tensor_reduce(
                    kp1_s[:],
                    kk_s[:].rearrange("p (w k) -> p w k", w=12),
                    mybir.AxisListType.X, Alu.max,
                )

                # ---- parity: compare even-chunk value ----
                i2_s = smpool.tile([C, 12], f32, tag="i2")
                nc.vector.tensor_scalar(
                    i2_s[:], kp1_s[:], 1024.0, -1024.0, Alu.mult, Alu.add
                )
                nc.vector.tensor_add(i2_s[:], i2_s[:], s_sl[:])
                i2u_s = smpool.tile([C, 12], dt.uint16, tag="i2u")
                nc.vector.tensor_copy(i2u_s[:], i2_s[:])
                g2_s = smpool.tile([C, 192], f16, tag="g2")
                nc.gpsimd.indirect_copy(g2_s[:], sbe[:], i2u_s[:], True)
                g2m_s = smpool.tile([C, 192], f16, tag="g2m")
                nc.vector.tensor_mul(g2m_s[:], g2_s[:], dsel_s[:, 0:192])
                ev_s = smpool.tile([C, 12], f32, tag="ev")
                nc.vector.tensor_reduce(
                    ev_s[:],
                    g2m_s[:].rearrange("p (w b) -> p w b", w=12),
                    mybir.AxisListType.X, Alu.add,
                )
                evq_s = smpool.tile([C, 12], f32, tag="evq")
                nc.vector.tensor_scalar(
                    evq_s[:], ev_s[:], LQ_SCALE, LQ_BIAS, Alu.mult, Alu.add
                )
                par_s = smpool.tile([C, 12], f32, tag="par")
                nc.vector.tensor_tensor(
                    par_s[:], evq_s[:], g0_s[:], Alu.not_equal
                )

                # ---- j, nc, e, e128r; stack (nc, e128r, w) [C, 36] ----
                stack_s = smpool.tile([C, 36], f32, tag="stack")
                j0_s = smpool.tile([C, 12], f32, tag="j0")
                nc.vector.scalar_tensor_tensor(
                    j0_s[:], par_s[:], 1024.0, s_sl[:], Alu.mult, Alu.add
                )
                jk_s = smpool.tile([C, 12], f32, tag="jk")
                nc.vector.tensor_scalar(
                    jk_s[:], kp1_s[:], 2048.0, -2048.0, Alu.mult, Alu.add
                )
                j_s = smpool.tile([C, 12], f32, tag="j")
                nc.vector.tensor_add(j_s[:], j0_s[:], jk_s[:])
                hf_s = smpool.tile([C, 12], f32, tag="hf")
                nc.vector.tensor_scalar(
                    hf_s[:], j_s[:], 1.0 / 2048.0, -0.49975, Alu.mult, Alu.add
                )
                e_s = smpool.tile([C, 12], f32, tag="e")
                nc.vector.tensor_scalar(
                    e_s[:], hf_s[:], MAGIC, MAGIC, Alu.add, Alu.subtract
                )
                # nc_w -> stack[:, 0:12]
                nc.vector.scalar_tensor_tensor(
                    stack_s[:, 0:12], e_s[:], -2048.0, j_s[:], Alu.mult, Alu.add
                )
                # e128r -> stack[:, 12:24]
                nc.vector.scalar_tensor_tensor(
                    stack_s[:, 12:24], e_s[:], 128.0,
                    rcon_s[:].to_broadcast([C, 12]),
                    Alu.mult, Alu.add,
                )
                # w -> stack[:, 24:36]
                nc.vector.tensor_scalar(
                    stack_s[:, 24:36], expv_s[:], rden_s[:], None, Alu.mult
                )

                # ---- transpose + DRAM bounce + wrapped/broadcast reads ----
                st_ps = ps_m.tile([36, 128], f32, tag="st")
                nc.tensor.transpose(st_ps[:], stack_s[:], ident_s[:])
                st16_s = smpool.tile([36, 128], f16, tag="st16")
                nc.scalar.copy(st16_s[:], st_ps[:])
                nc.sync.dma_start(scr_d[b][:], st16_s[:])

                wrow_s = midpool.tile([C, 1536], f16, tag="wrow")
                nc.sync.dma_start(
                    wrow_s[:],
                    scr_d[b][24:36]
                    .rearrange("t f -> (t f)")
                    .rearrange("(o x) -> o x", o=1)
                    .to_broadcast([C, 1536]),
                )
                i3f_s = smpool.tile([C, 96], f16, tag="i3f")
                i3f_src = (
                    scr_d[b][0:12]
                    .rearrange("t f -> (t f)")
                    .rearrange("(j v) -> v j", v=16)
                )
                for a8 in range(8):
                    nc.sync.dma_start(
                        i3f_s[16 * a8:16 * (a8 + 1), :], i3f_src
                    )
                i4f_s = smpool.tile([C, 96], f16, tag="i4f")
                i4f_src = (
                    scr_d[b][12:24]
                    .rearrange("t f -> (t f)")
                    .rearrange("(j v) -> v j", v=16)
                )
                for a8 in range(8):
                    nc.sync.dma_start(
                        i4f_s[16 * a8:16 * (a8 + 1), :], i4f_src
                    )
                i3_s = smpool.tile([C, 96], dt.int16, tag="i3")
                nc.vector.tensor_copy(i3_s[:], i3f_s[:])
                i4_s = smpool.tile([C, 96], dt.int16, tag="i4")
                nc.vector.tensor_copy(i4_s[:], i4f_s[:])

                # ---- gather comp columns, weight, scatter into M ----
                g3_s = midpool.tile([C, 1536], f32, tag="g3")
                nc.gpsimd.ap_gather(
                    g3_s[:], c32[:], i3_s[:],
                    channels=C, num_elems=NC, d=1, num_idxs=1536,
                )
                nc.vector.tensor_mul(
                    gwe_s[:].rearrange("p (l two) -> p l two", two=2)[:, :, 0],
                    g3_s[:], wrow_s[:],
                )
                m_s = midpool.tile([C, 2048], bf16, tag="M")
                nc.vector.memset(m_s[:], 0.0)
                nc.gpsimd.scatter_add(
                    m_s[:], i4_s[:], gwe_s[:],
                    channels=C, num_elems=1024, d=2, num_idxs=1536,
                )

                # ---- PV = sum_e Wv_e^T @ M_e ; diag extract ----
                pv_ps = ps_pv.tile([C, 128], f32, tag="pv")
                mv = m_s[:].rearrange("p (t two) -> p t two", two=2)
                for e in range(8):
                    nc.tensor.matmul(
                        pv_ps[:],
                        wv_s[:, e * C:(e + 1) * C],
                        mv[:, e * 128:(e + 1) * 128, 0],
                        start=(e == 0),
                        stop=(e == 7),
                    )
                pvh_s = smpool.tile([C, 128], f32, tag="pvh")
                nc.vector.tensor_mul(pvh_s[:], pv_ps[:], hrep_s[:])
                nc.vector.tensor_reduce(
                    pvt4_s[:, b::BPC],
                    pvh_s[:].rearrange("p (hb q) -> p q hb", hb=8),
                    mybir.AxisListType.X, Alu.add,
                )

            # ---- final projections ----
            o1_ps = ps_m.tile([C, 512], f32, tag="misc")
            for qi in range(NQ):
                nc.tensor.matmul(
                    o1_ps[:, 0:BPC],
                    wjwp_s[:, qi * C:(qi + 1) * C],
                    pvt4_s[:, qi * BPC:(qi + 1) * BPC],
                    start=(qi == 0),
                    stop=(qi == NQ - 1),
                )
            o2_s = smpool.tile([C, BPC], f32, tag="o2")
            nc.vector.tensor_add(o2_s[:], o1_ps[:, 0:BPC], xT_s[:])
            o3_ps = ps_m.tile([C, 512], f32, tag="misc")
            nc.tensor.matmul(o3_ps[0:BPC, 0:C], o2_s[:], wp_s[:])
            o4_s = smpool.tile([BPC, C], f32, tag="o4")
            nc.vector.tensor_add(o4_s[:], o3_ps[0:BPC, 0:C], bp4_s[:])
            nc.sync.dma_start(out_d[:], o4_s[:])

    nc.compile()
    return nc


def _host_prep(inputs):
    x = np.asarray(inputs["x"], dtype=np.float32)
    complement = np.asarray(inputs["complement"], np.float32)
    Wq = np.asarray(inputs["Wq"], np.float32)
    Wkv = np.asarray(inputs["Wkv"], np.float32)
    Wjw = np.asarray(inputs["Wjw"], np.float32)
    Wp = np.asarray(inputs["Wp"], np.float32)
    bp = np.asarray(inputs["bp"], np.float32)

    wkT = np.empty((C, 8 * C), np.float32)
    wv = np.empty((C, 8 * C), np.float32)
    for e in range(8):
        wkT[:, e * C:(e + 1) * C] = Wkv[:, e * 256: e * 256 + 128].T
        wv[:, e * C:(e + 1) * C] = Wkv[:, e * 256 + 128: e * 256 + 256]
    wjwp = (
        Wjw.reshape(H, NQ, HD, C).transpose(1, 0, 2, 3).reshape(NQ, C, C)
        .transpose(1, 0, 2).reshape(C, NQ * C)
    )
    bp4 = np.tile(bp.reshape(1, C), (BPC, 1)).astype(np.float32)
    hrep = np.kron(np.eye(H, dtype=np.float32), np.ones((HD, HD), np.float32))
    ident = np.eye(C, dtype=np.float32)
    srow = np.tile(np.arange(1024, dtype=np.float32).reshape(1, 1024), (C, 1))
    krow = np.tile(
        (np.tile(np.arange(8, dtype=np.float32), 12) * 1024).reshape(1, 96), (C, 1)
    )
    kval = np.tile(
        (np.tile(np.arange(8, dtype=np.float32), 12) + 1).reshape(1, 96), (C, 1)
    )
    dsel = np.zeros((C, 1536), np.float16)
    for p in range(C):
        dsel[p, (np.arange(96) * 16 + p % 16)] = 1.0
    rcon = np.arange(C, dtype=np.float32).reshape(C, 1)

    shared = dict(
        wq=np.ascontiguousarray(Wq),
        wkT=np.ascontiguousarray(wkT.astype(np.float16)),
        wv=np.ascontiguousarray(wv.astype(ml_dtypes.bfloat16)),
        wjwp=np.ascontiguousarray(wjwp),
        wp=np.ascontiguousarray(Wp),
        bp4=bp4,
        hrep=np.ascontiguousarray(hrep.astype(np.float16)),
        ident=ident,
        srow=srow,
        krow=np.ascontiguousarray(krow),
        kval=np.ascontiguousarray(kval),
        dsel=dsel,
        rcon=rcon,
    )

    in_maps = []
    for core in range(CORES):
        bs = range(core * BPC, (core + 1) * BPC)
        comp = np.stack(
            [
                np.concatenate([x[bb].reshape(1, C), complement[bb]], axis=0)
                for bb in bs
            ]
        ).astype(np.float32)
        compT = comp.transpose(0, 2, 1)
        m = dict(shared)
        m["c16h"] = np.ascontiguousarray(compT.astype(np.float16))
        m["c32"] = np.ascontiguousarray(compT)
        m["xT"] = np.ascontiguousarray(x[list(bs)].reshape(BPC, C).T)
        in_maps.append(m)
    return in_maps


def kernel(**inputs):
    from concourse.bass_utils import run_bass_kernel_spmd

    if "prog" not in _prog_cache:
        _prog_cache["prog"] = _build_program()
    nc = _prog_cache["prog"]

    in_maps = _host_prep(inputs)
    res = run_bass_kernel_spmd(nc, in_maps, core_ids=list(range(CORES)))
    out = np.empty((B, 1, C), np.float32)
    for core in range(CORES):
        o = res.results[core]["out"]
        for i in range(BPC):
            out[core * BPC + i, 0, :] = o[i]
    return out


if __name__ == "__main__":
    d = np.load("/root/problem/inputs_cache.npz")
    inputs = {k: d[k] for k in d.files if k != "ref_out"}
    ref = d["ref_out"]
    got = kernel(**inputs)
    err = np.abs(got - ref)
    print("absmax err:", err.max())
    print("Relative error:", err.max() / np.abs(ref).max())
    print("rel l2:", np.linalg.norm(got - ref) / np.linalg.norm(ref))


# revision 5
# speedup vs baseline: 2.3491x; 2.3491x over previous
"""Trainium2 Bass kernel for nn_MultiHeadCrossAttention (B=32, Nc=2048, H=8, topk=12).

D4 design: single-fp16-term S matmuls; ACT drains S chunks to fp16 SBUF;
DVE pair-max + 3-level tournament -> T[1024]; fp32 pack pm = q10*1024 + slot;
quarter max8 + match_replace rounds extract top-12 with indices; leaf/parity
resolved via group-redundant indirect_copy + mask-reduce; V never materialized:
gather comp columns (fp32), weight, scatter_add into M[(e,r)], finish with 8
accumulated matmuls Wv_e^T @ M_e, diagonal-extract via hrep mask.
"""

import sys
import numpy as np

for p in ("/opt/trn_rl_repo",):
    if p not in sys.path:
        sys.path.insert(0, p)

import ml_dtypes

B, CORES, BPC = 32, 8, 4
H, HD, NQ, TK, C, NC = 8, 16, 16, 12, 128, 2048
NJ = 8 * NC
MAGIC = 12582912.0          # 2^23 + 2^22
BIG = 3.0 * 2.0**32         # MAGIC * 1024: rounds fp32 to multiples of 1024
PACK_MUL = 8192.0 * 1024.0  # value quantum 1/8192 over range (-1, 1)
PACK_ADD = (8192.0 + MAGIC) * 1024.0
PACK_SUB = BIG
LQ_SCALE = 8192.0
LQ_BIAS = 8192.0 + MAGIC
NEG = -1e30

_prog_cache = {}


def _build_program():
    import concourse.bass as bass
    import concourse.mybir as mybir
    import concourse.tile as tile
    from concourse import bacc
    from concourse import library_config

    dt = mybir.dt
    Alu = mybir.AluOpType
    Act = mybir.ActivationFunctionType
    f32, f16, bf16 = dt.float32, dt.float16, dt.bfloat16
    nc = bacc.Bacc("TRN2", target_bir_lowering=False)

    c16h_d = nc.dram_tensor("c16h", [BPC, C, NC], f16, kind="ExternalInput")
    xT_d = nc.dram_tensor("xT", [C, BPC], f32, kind="ExternalInput")
    wq_d = nc.dram_tensor("wq", [C, 2048], f32, kind="ExternalInput")
    wkT_d = nc.dram_tensor("wkT", [C, 8 * C], f16, kind="ExternalInput")
    wv_d = nc.dram_tensor("wv", [C, 8 * C], f16, kind="ExternalInput")
    wjwp_d = nc.dram_tensor("wjwp", [C, NQ * C], f32, kind="ExternalInput")
    wp_d = nc.dram_tensor("wp", [C, C], f32, kind="ExternalInput")
    bp4_d = nc.dram_tensor("bp4", [BPC, C], f32, kind="ExternalInput")
    hrep_d = nc.dram_tensor("hrep", [C, C], f16, kind="ExternalInput")
    me_d = nc.dram_tensor("me", [C, 512], f32, kind="ExternalInput")
    mo_d = nc.dram_tensor("mo", [C, 512], f32, kind="ExternalInput")
    srow_d = nc.dram_tensor("srow", [C, 1024], f32, kind="ExternalInput")
    krow_d = nc.dram_tensor("krow", [C, 96], f32, kind="ExternalInput")
    kval_d = nc.dram_tensor("kval", [C, 96], f32, kind="ExternalInput")
    dsel_d = nc.dram_tensor("dsel", [C, 1536], f16, kind="ExternalInput")
    out_d = nc.dram_tensor("out", [BPC, C], f32, kind="ExternalOutput")

    with tile.TileContext(nc) as tc:
        nc.gpsimd.load_library(library_config.ap_gather)
        with (
            tc.tile_pool(name="weights", bufs=1) as wpool,
            tc.tile_pool(name="inb", bufs=2) as inpool,       # c16h, c32
            tc.tile_pool(name="sbig", bufs=2) as sbig,        # SbEven, R
            tc.tile_pool(name="stage", bufs=3) as stpool,     # odd chunks
            tc.tile_pool(name="mid", bufs=1) as midpool,      # L1, L2, pm, G, M...
            tc.tile_pool(name="small", bufs=2) as smpool,     # winner stage tiles
            tc.tile_pool(name="ps_s", bufs=2, space="PSUM") as ps_s,
            tc.tile_pool(name="ps_a", bufs=1, space="PSUM") as ps_a,
            tc.tile_pool(name="ps_m", bufs=1, space="PSUM") as ps_m,
        ):
            # ---- resident weights/constants ----
            wq_s = wpool.tile([C, 2048], f32)
            nc.sync.dma_start(wq_s[:], wq_d[:])
            wkT_s = wpool.tile([C, 8 * C], f16)
            nc.sync.dma_start(wkT_s[:], wkT_d[:])
            wv_s = wpool.tile([C, 8 * C], f16)
            nc.sync.dma_start(wv_s[:], wv_d[:])
            wjwp_s = wpool.tile([C, NQ * C], f32)
            nc.sync.dma_start(wjwp_s[:], wjwp_d[:])
            wp_s = wpool.tile([C, C], f32)
            nc.sync.dma_start(wp_s[:], wp_d[:])
            bp4_s = wpool.tile([BPC, C], f32)
            nc.sync.dma_start(bp4_s[:], bp4_d[:])
            hrep_s = wpool.tile([C, C], f16)
            nc.sync.dma_start(hrep_s[:], hrep_d[:])
            me_s = wpool.tile([C, 512], f32)
            nc.sync.dma_start(me_s[:], me_d[:])
            mo_s = wpool.tile([C, 512], f32)
            nc.sync.dma_start(mo_s[:], mo_d[:])
            srow_s = wpool.tile([C, 1024], f32)
            nc.sync.dma_start(srow_s[:], srow_d[:])
            krow_s = wpool.tile([C, 96], f32)
            nc.sync.dma_start(krow_s[:], krow_d[:])
            kval_s = wpool.tile([C, 96], f32)
            nc.sync.dma_start(kval_s[:], kval_d[:])
            dsel_s = wpool.tile([C, 1536], f16)
            nc.sync.dma_start(dsel_s[:], dsel_d[:])
            xT_s = wpool.tile([C, BPC], f32)
            nc.sync.dma_start(xT_s[:], xT_d[:])

            bm1_s = wpool.tile([C, 1], f32)      # exp bias constant
            nc.vector.memset(bm1_s[:], -1.0)

            # ---- Q projection for all batches: qt [(h,hd), (q,b)] ----
            qt_ps = ps_m.tile([C, 512], f32, tag="misc")
            for qi in range(NQ):
                nc.tensor.matmul(
                    qt_ps[:, qi * BPC:(qi + 1) * BPC],
                    wq_s[:, qi * C:(qi + 1) * C],
                    xT_s[:],
                )
            qt_s = wpool.tile([C, NQ * BPC], f32)
            nc.scalar.copy(qt_s[:], qt_ps[:, : NQ * BPC])

            pvt4_s = wpool.tile([C, NQ * BPC], f32)

            for b in range(BPC):
                c16 = inpool.tile([C, NC], f16, tag="c16")
                nc.sync.dma_start(c16[:], c16h_d[b])

                # ---- qbd (block diag, 0.25 scale), fp16 ----
                qfull_s = smpool.tile([C, C], f32, tag="qfull")
                qsl = (
                    qt_s[:, b::BPC]
                    .rearrange("p (o q) -> p o q", o=1)
                    .to_broadcast([C, H, NQ])
                )
                nc.vector.tensor_scalar(
                    qfull_s[:].rearrange("p (o q) -> p o q", o=H),
                    qsl, 0.25, None, Alu.mult,
                )
                qbd_s = smpool.tile([C, C], f16, tag="qbd")
                nc.vector.tensor_mul(qbd_s[:], qfull_s[:], hrep_s[:])

                # ---- A_e [c, row] fp16 ----
                a16 = smpool.tile([C, 8 * C], f16, tag="a16")
                for half in range(2):
                    a_ps = ps_a.tile([C, 512], f32, tag="a")
                    for i in range(4):
                        e = half * 4 + i
                        nc.tensor.matmul(
                            a_ps[:, i * C:(i + 1) * C],
                            wkT_s[:, e * C:(e + 1) * C],
                            qbd_s[:],
                        )
                    nc.scalar.copy(a16[:, half * 512:(half + 1) * 512], a_ps[:])

                # ---- S chunks -> ACT fp16 copies; L0 pair-max -> R ----
                sbe = sbig.tile([C, 8192], f16, tag="sbe")   # even chunks
                r_s = sbig.tile([C, 8192], f16, tag="R")
                for k in range(8):   # chunk pair k: chunks 2k (even), 2k+1 (odd)
                    odd = stpool.tile([C, 1024], f16, tag="odd")
                    for ch in (2 * k, 2 * k + 1):
                        e, half = ch // 2, ch % 2
                        s_ps = ps_s.tile([C, 1024], f32, tag="s")
                        for n2 in range(2):
                            col = half * 1024 + n2 * 512
                            nc.tensor.matmul(
                                s_ps[:, n2 * 512:(n2 + 1) * 512],
                                a16[:, e * C:(e + 1) * C],
                                c16[:, col:col + 512],
                            )
                        dst = sbe[:, k * 1024:(k + 1) * 1024] if ch % 2 == 0 else odd[:]
                        nc.scalar.copy(dst, s_ps[:])
                    nc.vector.tensor_max(
                        r_s[:, k * 1024:(k + 1) * 1024],
                        sbe[:, k * 1024:(k + 1) * 1024],
                        odd[:],
                    )

                # ---- V table: vt[(h,hd), j] bf16 ----
                vt_s = midpool.tile([C, NJ], bf16, tag="VT")
                for ch in range(16):
                    e, half = ch // 2, ch % 2
                    v_ps = ps_s.tile([C, 1024], f32, tag="s")
                    for n2 in range(2):
                        col = half * 1024 + n2 * 512
                        nc.tensor.matmul(
                            v_ps[:, n2 * 512:(n2 + 1) * 512],
                            wv_s[:, e * C:(e + 1) * C],
                            c16[:, col:col + 512],
                        )
                    dst = vt_s[:, ch * 1024:(ch + 1) * 1024]
                    if ch < 12:
                        nc.scalar.copy(dst, v_ps[:])
                    else:
                        nc.vector.tensor_copy(dst, v_ps[:])

                # ---- tournament: R [p,8,1024] -> T [p,1024] ----
                l1_s = midpool.tile([C, 4096], f16, tag="L1")
                rv = r_s[:].rearrange("p (k f) -> p k f", k=8)
                nc.vector.tensor_max(
                    l1_s[:].rearrange("p (k f) -> p k f", k=4),
                    rv[:, 0:4, :], rv[:, 4:8, :],
                )
                l2_s = midpool.tile([C, 2048], f16, tag="L2")
                l1v = l1_s[:].rearrange("p (k f) -> p k f", k=4)
                nc.vector.tensor_max(
                    l2_s[:].rearrange("p (k f) -> p k f", k=2),
                    l1v[:, 0:2, :], l1v[:, 2:4, :],
                )
                t_s = midpool.tile([C, 1024], f16, tag="T")
                nc.vector.tensor_max(t_s[:], l2_s[:, 0:1024], l2_s[:, 1024:2048])

                # ---- pack pm = q10*1024 + s (exact fp32 ints) ----
                t1_s = midpool.tile([C, 1024], f32, tag="t1")
                nc.vector.tensor_scalar(
                    t1_s[:], t_s[:], PACK_MUL, PACK_ADD, Alu.mult, Alu.add
                )
                pm_s = midpool.tile([C, 1024], f32, tag="pm")
                nc.vector.scalar_tensor_tensor(
                    pm_s[:], t1_s[:], PACK_SUB, srow_s[:], Alu.subtract, Alu.add
                )

                # ---- quarter extract -> 32 cands -> top8 + next4 ----
                cand_s = smpool.tile([C, 32], f32, tag="cand")
                for qd in range(4):
                    nc.vector.max(
                        cand_s[:, qd * 8:(qd + 1) * 8],
                        pm_s[:, qd * 256:(qd + 1) * 256],
                    )
                t8a = smpool.tile([C, 8], f32, tag="t8a")
                nc.vector.max(t8a[:], cand_s[:])
                c2_s = smpool.tile([C, 32], f32, tag="c2")
                nc.vector.match_replace(c2_s[:], t8a[:], cand_s[:], NEG)
                t8b = smpool.tile([C, 8], f32, tag="t8b")
                nc.vector.max(t8b[:], c2_s[:])
                pw_s = smpool.tile([C, 12], f32, tag="pw")
                nc.vector.tensor_copy(pw_s[:, 0:8], t8a[:])
                nc.vector.tensor_copy(pw_s[:, 8:12], t8b[:, 0:4])

                # ---- decode: r1 = round1024(pm); s = pm - r1 (mod fix); qv ----
                r1_s = smpool.tile([C, 12], f32, tag="r1")
                nc.vector.tensor_scalar(
                    r1_s[:], pw_s[:], BIG, BIG, Alu.add, Alu.subtract
                )
                sp_s = smpool.tile([C, 12], f32, tag="sp")
                nc.vector.tensor_sub(sp_s[:], pw_s[:], r1_s[:])
                neg_s = smpool.tile([C, 12], f32, tag="neg")
                nc.vector.tensor_scalar(neg_s[:], sp_s[:], 0.0, None, Alu.is_lt)
                s_sl = smpool.tile([C, 12], f32, tag="s")
                nc.vector.scalar_tensor_tensor(
                    s_sl[:], neg_s[:], 1024.0, sp_s[:], Alu.mult, Alu.add
                )
                qv_s = smpool.tile([C, 12], f32, tag="qv")
                nc.vector.scalar_tensor_tensor(
                    qv_s[:], r1_s[:], 1.0 / 1024.0, neg_s[:], Alu.mult, Alu.subtract
                )
                g0_s = smpool.tile([C, 12], f32, tag="g0")
                nc.vector.tensor_scalar(g0_s[:], qv_s[:], MAGIC, None, Alu.add)

                # ---- softmax weights from quantized values ----
                expv_s = smpool.tile([C, 12], f32, tag="expv")
                nc.scalar.activation(
                    expv_s[:], qv_s[:], Act.Exp, bias=bm1_s[:], scale=1.0 / 8192.0
                )
                den_s = smpool.tile([C, 1], f32, tag="den")
                nc.vector.tensor_reduce(
                    den_s[:], expv_s[:], mybir.AxisListType.X, Alu.add
                )
                rden_s = smpool.tile([C, 1], f32, tag="rden")
                nc.vector.reciprocal(rden_s[:], den_s[:])

                # ---- leaf resolve: gather 8 leaves/winner (group redundant) ----
                i1_s = smpool.tile([C, 96], f32, tag="i1")
                nc.vector.tensor_add(
                    i1_s[:].rearrange("p (w k) -> p w k", w=12),
                    krow_s[:].rearrange("p (w k) -> p w k", w=12),
                    s_sl[:].rearrange("p (w o) -> p w o", o=1).to_broadcast([C, 12, 8]),
                )
                i1u_s = smpool.tile([C, 96], dt.uint16, tag="i1u")
                nc.vector.tensor_copy(i1u_s[:], i1_s[:])
                g1_s = midpool.tile([C, 1536], f16, tag="g1")
                nc.gpsimd.indirect_copy(
                    g1_s[:, 0:768], r_s[:], i1u_s[:, 0:48], True
                )
                nc.gpsimd.indirect_copy(
                    g1_s[:, 768:1536], r_s[:], i1u_s[:, 48:96], True
                )
                g1m_s = midpool.tile([C, 1536], f16, tag="g1m")
                nc.vector.tensor_mul(g1m_s[:], g1_s[:], dsel_s[:])
                diag_s = smpool.tile([C, 96], f32, tag="diag")
                nc.vector.tensor_reduce(
                    diag_s[:],
                    g1m_s[:].rearrange("p (t b) -> p t b", t=96),
                    mybir.AxisListType.X, Alu.add,
                )
                lq_s = smpool.tile([C, 96], f32, tag="lq")
                nc.vector.tensor_scalar(
                    lq_s[:], diag_s[:], LQ_SCALE, LQ_BIAS, Alu.mult, Alu.add
                )
                eq_s = smpool.tile([C, 96], f32, tag="eq")
                nc.vector.tensor_tensor(
                    eq_s[:].rearrange("p (w k) -> p w k", w=12),
                    lq_s[:].rearrange("p (w k) -> p w k", w=12),
                    g0_s[:].rearrange("p (w o) -> p w o", o=1).to_broadcast([C, 12, 8]),
                    Alu.is_equal,
                )
                kk_s = smpool.tile([C, 96], f32, tag="kk")
                nc.vector.tensor_mul(kk_s[:], eq_s[:], kval_s[:])
                kp1_s = smpool.tile([C, 12], f32, tag="kp1")
                nc.vector.tensor_reduce(
                    kp1_s[:],
                    kk_s[:].rearrange("p (w k) -> p w k", w=12),
                    mybir.AxisListType.X, Alu.max,
                )

                # ---- parity: compare even-chunk value ----
                i2_s = smpool.tile([C, 12], f32, tag="i2")
                nc.vector.tensor_scalar(
                    i2_s[:], kp1_s[:], 1024.0, -1024.0, Alu.mult, Alu.add
                )
                nc.vector.tensor_add(i2_s[:], i2_s[:], s_sl[:])
                i2u_s = smpool.tile([C, 12], dt.uint16, tag="i2u")
                nc.vector.tensor_copy(i2u_s[:], i2_s[:])
                g2_s = smpool.tile([C, 192], f16, tag="g2")
                nc.gpsimd.indirect_copy(g2_s[:], sbe[:], i2u_s[:], True)
                g2m_s = smpool.tile([C, 192], f16, tag="g2m")
                nc.vector.tensor_mul(g2m_s[:], g2_s[:], dsel_s[:, 0:192])
                ev_s = smpool.tile([C, 12], f32, tag="ev")
                nc.vector.tensor_reduce(
                    ev_s[:],
                    g2m_s[:].rearrange("p (w b) -> p w b", w=12),
                    mybir.AxisListType.X, Alu.add,
                )
                evq_s = smpool.tile([C, 12], f32, tag="evq")
                nc.vector.tensor_scalar(
                    evq_s[:], ev_s[:], LQ_SCALE, LQ_BIAS, Alu.mult, Alu.add
                )
                par_s = smpool.tile([C, 12], f32, tag="par")
                nc.vector.tensor_tensor(
                    par_s[:], evq_s[:], g0_s[:], Alu.not_equal
                )

                # ---- j (global winner index) ----
                j0_s = smpool.tile([C, 12], f32, tag="j0")
                nc.vector.scalar_tensor_tensor(
                    j0_s[:], par_s[:], 1024.0, s_sl[:], Alu.mult, Alu.add
                )
                jk_s = smpool.tile([C, 12], f32, tag="jk")
                nc.vector.tensor_scalar(
                    jk_s[:], kp1_s[:], 2048.0, -2048.0, Alu.mult, Alu.add
                )
                j_s = smpool.tile([C, 12], f32, tag="j")
                nc.vector.tensor_add(j_s[:], j0_s[:], jk_s[:])

                # ---- pair idx gp = floor(j/2), pair parity; padded to 16 ----
                gp_s = smpool.tile([C, 16], f32, tag="gp")
                nc.vector.memset(gp_s[:], 0.0)
                nc.vector.tensor_scalar(
                    gp_s[:, 0:12], j_s[:], 0.5, -0.25, Alu.mult, Alu.add
                )
                nc.vector.tensor_scalar(
                    gp_s[:, 0:12], gp_s[:, 0:12], MAGIC, MAGIC, Alu.add, Alu.subtract
                )
                gp_i = smpool.tile([C, 16], dt.int16, tag="gpi")
                nc.vector.tensor_copy(gp_i[:], gp_s[:])
                par2_s = smpool.tile([C, 16], f32, tag="par2")
                nc.vector.memset(par2_s[:], 0.0)
                nc.vector.scalar_tensor_tensor(
                    par2_s[:, 0:12], gp_s[:, 0:12], -2.0, j_s[:], Alu.mult, Alu.add
                )
                wn_s = smpool.tile([C, 16], f32, tag="wn")
                nc.vector.memset(wn_s[:], 0.0)
                nc.vector.tensor_scalar(
                    wn_s[:, 0:12], expv_s[:], rden_s[:], None, Alu.mult
                )
                wnE_s = smpool.tile([C, 16], f32, tag="wnE")
                nc.vector.scalar_tensor_tensor(
                    wnE_s[:], par2_s[:], -1.0, wn_s[:], Alu.mult, Alu.mult
                )
                nc.vector.tensor_add(wnE_s[:], wnE_s[:], wn_s[:])
                wnO_s = smpool.tile([C, 16], f32, tag="wnO")
                nc.vector.tensor_mul(wnO_s[:], wn_s[:], par2_s[:])

                # ---- gather V pairs (per-head core lists) ----
                g_s = smpool.tile([C, 512], bf16, tag="G")
                nc.gpsimd.ap_gather(
                    g_s[:], vt_s[:], gp_i[:],
                    channels=C, num_elems=NJ // 2, d=2, num_idxs=256,
                )

                # ---- weights -> [(h,d), (i,q,parity)] via headrep matmul ----
                wEb = (
                    wnE_s[:].rearrange("p (i o) -> p i o", o=1)
                    .to_broadcast([C, NQ, 32])
                )
                wOb = (
                    wnO_s[:].rearrange("p (i o) -> p i o", o=1)
                    .to_broadcast([C, NQ, 32])
                )
                tmpE = smpool.tile([C, 512], f16, tag="tmpE")
                nc.vector.tensor_mul(
                    tmpE[:].rearrange("p (i s) -> p i s", s=32),
                    wEb,
                    me_s[:].rearrange("p (i s) -> p i s", s=32),
                )
                wsc = smpool.tile([C, 512], f16, tag="wsc")
                nc.vector.tensor_mul(
                    wsc[:].rearrange("p (i s) -> p i s", s=32),
                    wOb,
                    mo_s[:].rearrange("p (i s) -> p i s", s=32),
                )
                nc.vector.tensor_add(wsc[:], wsc[:], tmpE[:])
                wb_ps = ps_m.tile([C, 512], f32, tag="misc")
                nc.tensor.matmul(wb_ps[:], hrep_s[:], wsc[:])
                wb_s = smpool.tile([C, 512], bf16, tag="wb")
                nc.scalar.copy(wb_s[:], wb_ps[:])

                gw_s = smpool.tile([C, 512], f32, tag="gw")
                nc.vector.tensor_mul(gw_s[:], g_s[:], wb_s[:])
                nc.vector.tensor_reduce(
                    pvt4_s[:, b::BPC],
                    gw_s[:].rearrange("p (i q r) -> p q i r", q=NQ, r=2),
                    mybir.AxisListType.XY,
                    Alu.add,
                )

            # ---- final projections ----
            o1_ps = ps_m.tile([C, 512], f32, tag="misc")
            for qi in range(NQ):
                nc.tensor.matmul(
                    o1_ps[:, 0:BPC],
                    wjwp_s[:, qi * C:(qi + 1) * C],
                    pvt4_s[:, qi * BPC:(qi + 1) * BPC],
                    start=(qi == 0),
                    stop=(qi == NQ - 1),
                )
            o2_s = smpool.tile([C, BPC], f32, tag="o2")
            nc.vector.tensor_add(o2_s[:], o1_ps[:, 0:BPC], xT_s[:])
            o3_ps = ps_m.tile([C, 512], f32, tag="misc")
            nc.tensor.matmul(o3_ps[0:BPC, 0:C], o2_s[:], wp_s[:])
            o4_s = smpool.tile([BPC, C], f32, tag="o4")
            nc.vector.tensor_add(o4_s[:], o3_ps[0:BPC, 0:C], bp4_s[:])
            nc.sync.dma_start(out_d[:], o4_s[:])

    nc.compile()
    return nc


def _host_prep(inputs):
    x = np.asarray(inputs["x"], dtype=np.float32)
    complement = np.asarray(inputs["complement"], np.float32)
    Wq = np.asarray(inputs["Wq"], np.float32)
    Wkv = np.asarray(inputs["Wkv"], np.float32)
    Wjw = np.asarray(inputs["Wjw"], np.float32)
    Wp = np.asarray(inputs["Wp"], np.float32)
    bp = np.asarray(inputs["bp"], np.float32)

    wkT = np.empty((C, 8 * C), np.float32)
    wv = np.empty((C, 8 * C), np.float32)
    for e in range(8):
        wkT[:, e * C:(e + 1) * C] = Wkv[:, e * 256: e * 256 + 128].T
        wv[:, e * C:(e + 1) * C] = Wkv[:, e * 256 + 128: e * 256 + 256]
    wjwp = (
        Wjw.reshape(H, NQ, HD, C).transpose(1, 0, 2, 3).reshape(NQ, C, C)
        .transpose(1, 0, 2).reshape(C, NQ * C)
    )
    bp4 = np.tile(bp.reshape(1, C), (BPC, 1)).astype(np.float32)
    hrep = np.kron(np.eye(H, dtype=np.float32), np.ones((HD, HD), np.float32))
    s_idx = np.tile(np.arange(32).reshape(1, 1, 32), (C, NQ, 1))
    p_idx = (np.arange(C) % NQ).reshape(C, 1, 1)
    me = (s_idx == 2 * p_idx).astype(np.float32).reshape(C, 512)
    mo = (s_idx == 2 * p_idx + 1).astype(np.float32).reshape(C, 512)
    srow = np.tile(np.arange(1024, dtype=np.float32).reshape(1, 1024), (C, 1))
    krow = np.tile(
        (np.tile(np.arange(8, dtype=np.float32), 12) * 1024).reshape(1, 96), (C, 1)
    )
    kval = np.tile(
        (np.tile(np.arange(8, dtype=np.float32), 12) + 1).reshape(1, 96), (C, 1)
    )
    dsel = np.zeros((C, 1536), np.float16)
    for p in range(C):
        dsel[p, (np.arange(96) * 16 + p % 16)] = 1.0

    shared = dict(
        wq=np.ascontiguousarray(Wq),
        wkT=np.ascontiguousarray(wkT.astype(np.float16)),
        wv=np.ascontiguousarray(wv.astype(np.float16)),
        wjwp=np.ascontiguousarray(wjwp),
        wp=np.ascontiguousarray(Wp),
        bp4=bp4,
        hrep=np.ascontiguousarray(hrep.astype(np.float16)),
        me=np.ascontiguousarray(me),
        mo=np.ascontiguousarray(mo),
        srow=srow,
        krow=np.ascontiguousarray(krow),
        kval=np.ascontiguousarray(kval),
        dsel=dsel,
    )

    in_maps = []
    for core in range(CORES):
        bs = range(core * BPC, (core + 1) * BPC)
        comp = np.stack(
            [
                np.concatenate([x[bb].reshape(1, C), complement[bb]], axis=0)
                for bb in bs
            ]
        ).astype(np.float32)
        compT = comp.transpose(0, 2, 1)
        m = dict(shared)
        m["c16h"] = np.ascontiguousarray(compT.astype(np.float16))
        m["xT"] = np.ascontiguousarray(x[list(bs)].reshape(BPC, C).T)
        in_maps.append(m)
    return in_maps


def kernel(**inputs):
    from concourse.bass_utils import run_bass_kernel_spmd

    if "prog" not in _prog_cache:
        _prog_cache["prog"] = _build_program()
    nc = _prog_cache["prog"]

    in_maps = _host_prep(inputs)
    res = run_bass_kernel_spmd(nc, in_maps, core_ids=list(range(CORES)))
    out = np.empty((B, 1, C), np.float32)
    for core in range(CORES):
        o = res.results[core]["out"]
        for i in range(BPC):
            out[core * BPC + i, 0, :] = o[i]
    return out


if __name__ == "__main__":
    d = np.load("/root/problem/inputs_cache.npz")
    inputs = {k: d[k] for k in d.files if k != "ref_out"}
    ref = d["ref_out"]
    got = kernel(**inputs)
    err = np.abs(got - ref)
    print("absmax err:", err.max())
    print("Relative error:", err.max() / np.abs(ref).max())
    print("rel l2:", np.linalg.norm(got - ref) / np.linalg.norm(ref))
